# revision 38
# baseline (speedup 1.0000x reference)
"""MoE grouped-experts kernel for Trainium2 (8 NeuronCores, expert-parallel).

Strategy
--------
Expert-parallel: 32 experts packed onto 8 cores x 4 slots. Routing
(sort-by-expert, capacity truncation at the reference's C=1024) is computed
on host from the tiny `indices` tensor; token rows are gathered per expert,
zero-padded to the slot's streamed length, and pre-transposed so the device
kernel is a pure stream of bf16 matmuls (fp32 PSUM accumulation) with zero
on-device transposes:

  GEMM1 (h^T orientation):  hT[m,c] = sum_k gup[k,m] * xT[k,c]
        stationary = gup tile [128 D-rows, 128 cols-of-2I], moving = xT
  act:  aT = silu(1.702*min(gate,7)) * (clip(up,-7,7)+1)   (the 1/1.702 is
        folded into the routing probs applied in the host-side combine)
  GEMM2 (y^T orientation):  yT[d,c] = sum_k down[k,d] * aT[k,c]
        stationary = natural down chunk [128 I-rows, 128 D-cols],
        moving = aT tokens.  Output leaves the device transposed
        [16 d-chunks, 128, tokens]; the host combine untransposes.

Per-slot token streams are trimmed to the max *actual* expert load in the
slot (padded to a multiple of 16), not to 128-row blocks: streamed columns
beyond an expert's load are zeros on host, flow through as zeros, and are
never gathered.  Routing probs (and the 1/1.702 silu fold) are applied on
the host during the combine, so the device emits unscaled yT.  All operands
are bf16 (PSUM accumulates fp32), halving HBM traffic and SBUF pressure
versus fp32.

DMA pacing: the sync ring is a single FIFO fanned over 16 engines, so
every transfer is issued in need-order and sized so nothing blocks a
latency-critical load: xT is split per contraction tile (139 KB), gup
stationaries are issued two (i,half) steps ahead across slot boundaries,
and the next slot's xT is paced between GEMM2's down-chunk loads.
"""

import math
from contextlib import ExitStack

import numpy as np
import ml_dtypes

BF16 = ml_dtypes.bfloat16

N_TOKENS, DIM = 4096, 2048
N_EXPERTS, TOPK, INTER = 32, 4, 1408
ALPHA, LIMIT, LIN_OFFSET = 1.702, 7.0, 1.0

NCORE = 8
NSLOT = N_EXPERTS // NCORE        # expert slots per core = 4
KD = DIM // 128                   # 16 contraction tiles for GEMM1
KI = INTER // 128                 # 11 contraction tiles for GEMM2
NDC = DIM // 128                  # 16 output d-chunks for GEMM2
C_REF = 2 * ((N_TOKENS * TOPK + N_EXPERTS - 1) // N_EXPERTS)  # 1024
XT_SLABS = ((2, 2, 4, 8),) + ((8, 8),) * (NSLOT - 1)  # k-tiles per xT DMA slab

_PROG_CACHE: dict = {}


def _groups(lpad: int):
    """Split lpad into PSUM-bank-sized moving groups (<=512, mult of 2)."""
    ng = max(1, math.ceil(lpad / 512))
    per = (lpad // ng // 2) * 2
    sizes = [per] * (ng - 1) + [lpad - per * (ng - 1)]
    assert all(0 < s <= 512 for s in sizes), sizes
    out, off = [], 0
    for s in sizes:
        out.append((off, s))
        off += s
    return out


def _build_program(lpads: tuple):
    import concourse.bacc as bacc
    import concourse.mybir as mybir
    import concourse.tile as tile
    from concourse.alu_op_type import AluOpType

    F32 = mybir.dt.float32
    BF = mybir.dt.bfloat16
    TOT = sum(lpads)
    lmax = max(lpads)
    toff = np.concatenate([[0], np.cumsum(lpads)]).tolist()
    xt_sizes = [KD * 128 * lp for lp in lpads]
    xt_off = np.concatenate([[0], np.cumsum(xt_sizes)]).tolist()
    # slot 0 streams in fine slabs (fast first matmul); rest in halves
    slab_k0 = [np.concatenate([[0], np.cumsum(sk)]).tolist() for sk in XT_SLABS]

    nc = bacc.Bacc(None, target_bir_lowering=False, debug=False)
    with ExitStack() as ctx:
        tc = ctx.enter_context(tile.TileContext(nc))
        dram = ctx.enter_context(tc.tile_pool(name="dram", bufs=1, space="DRAM"))
        xt_d = dram.tile([xt_off[-1]], BF, kind="ExternalInput")
        gup_d = dram.tile([NSLOT, 2, KI, 128, KD * 128], BF, kind="ExternalInput")
        down_d = dram.tile([NSLOT, NDC // 2, 128, 2 * KI * 128], BF,
                           kind="ExternalInput")
        y_d = dram.tile([NDC, 128, TOT], F32, kind="ExternalOutput")
        names = {"xt": xt_d.name, "gup": gup_d.name, "down": down_d.name,
                 "y": y_d.name}

        xtq_pool = ctx.enter_context(tc.tile_pool(name="xtq", bufs=4))
        xth_pool = ctx.enter_context(tc.tile_pool(name="xth", bufs=4))
        gup_pool = ctx.enter_context(tc.tile_pool(name="gup", bufs=6))
        gup0_pool = ctx.enter_context(tc.tile_pool(name="gup0", bufs=4))
        down_pool = ctx.enter_context(tc.tile_pool(name="down", bufs=4))
        at_pool = ctx.enter_context(tc.tile_pool(name="at", bufs=2))
        fg_pool = ctx.enter_context(tc.tile_pool(name="fg", bufs=3))
        tmp_pool = ctx.enter_context(tc.tile_pool(name="tmp", bufs=4))
        y_pool = ctx.enter_context(tc.tile_pool(name="yt", bufs=4))
        psg = ctx.enter_context(tc.tile_pool(name="psg", bufs=8, space="PSUM"))

        def load_xt_slab(j, si):
            """One contiguous p-major slab of several k-tiles of slot j's xT."""
            lp = lpads[j]
            nk = XT_SLABS[j][si]
            pool, tag = (xtq_pool, "xtq") if j == 0 else (xth_pool, "xth")
            t = pool.tile([128, 8 * lmax], BF, tag=tag)
            a = xt_off[j] + 128 * slab_k0[j][si] * lp
            src = xt_d[a: a + 128 * nk * lp]
            nc.sync.dma_start(out=t[:, :nk * lp],
                              in_=src.rearrange("(p c) -> p c", p=128))
            return t

        # flat (j, i, half) order of gup stationary loads, prefetched depth-2
        gsteps = [(j, i, half)
                  for j in range(NSLOT) for i in range(KI) for half in (0, 1)]
        gup_tiles: dict = {}

        def issue_gup(s):
            if s < len(gsteps):
                j, i, half = gsteps[s]
                t = gup_pool.tile([128, KD * 128], BF, tag="gup")
                nc.sync.dma_start(out=t[:], in_=gup_d[j, half, i])
                gup_tiles[s] = t

        # first two stationaries are split in k-halves, interleaved with the
        # xT tiles in consumption order, so matmul k=0 starts early and the
        # first two sweeps ride just behind the DMA ring
        def issue_gup_split(s):
            j, i, half = gsteps[s]
            hw_ = (KD // 2) * 128
            ha = gup0_pool.tile([128, hw_], BF, tag="gup0")
            nc.sync.dma_start(out=ha[:], in_=gup_d[j, half, i, :, :hw_])
            return ha

        def issue_gup_split2(s, ha):
            j, i, half = gsteps[s]
            hw_ = (KD // 2) * 128
            hb = gup0_pool.tile([128, hw_], BF, tag="gup0")
            nc.sync.dma_start(out=hb[:], in_=gup_d[j, half, i, :, hw_:])
            gup_tiles[s] = (ha, hb)

        _h0 = issue_gup_split(0)
        xt_tiles = [load_xt_slab(0, 0)]
        issue_gup_split2(0, _h0)
        xt_tiles.append(load_xt_slab(0, 1))
        _h1 = issue_gup_split(1)
        xt_tiles.append(load_xt_slab(0, 2))
        issue_gup_split2(1, _h1)
        xt_tiles.append(load_xt_slab(0, 3))

        down_tiles: dict = {}

        def issue_down(j, pd):
            """Load pair pd = down chunks (2pd, 2pd+1) in one transfer."""
            t = down_pool.tile([128, 2 * KI * 128], BF, tag="down")
            nc.sync.dma_start(out=t[:], in_=down_d[j, pd])
            down_tiles[pd] = t

        for j in range(NSLOT):
            LP = lpads[j]
            groups = _groups(LP)
            xt_h = xt_tiles
            next_xt: list = []
            kmap = [(si, r) for si, nk in enumerate(XT_SLABS[j])
                    for r in range(nk)]

            def xt_ap(k, g0, gw, LP=LP, xt_h=xt_h, kmap=kmap):
                si, r = kmap[k]
                return xt_h[si][:, r * LP + g0: r * LP + g0 + gw]

            at_sb = at_pool.tile([128, KI * lmax], BF, tag="at")

            for i in range(KI):
                for half in (0, 1):  # 0 = gate, 1 = up
                    s = (j * KI + i) * 2 + half
                    issue_gup(s + 2)
                    if half == 1 and j + 1 < NSLOT and i in (4, 7):
                        # pace next slot's xT through GEMM1's ring slack
                        next_xt.append(load_xt_slab(j + 1, len(next_xt)))
                    if i == KI - 2 and half == 1:
                        issue_down(j, 0)
                    gsb = gup_tiles.pop(s)
                    if isinstance(gsb, tuple):
                        def g_ap(k, gsb=gsb):
                            h = KD // 2
                            return gsb[k // h][:, (k % h) * 128:(k % h + 1) * 128]
                    else:
                        def g_ap(k, gsb=gsb):
                            return gsb[:, k * 128:(k + 1) * 128]
                    pss = [psg.tile([128, 512], F32, tag="ps",
                                    name=f"ps1_{j}_{i}_{half}_{gi}")
                           for gi in range(len(groups))]
                    for k in range(KD):
                        for gi, (g0, gw) in enumerate(groups):
                            nc.tensor.matmul(
                                pss[gi][:, :gw],
                                lhsT=g_ap(k),
                                rhs=xt_ap(k, g0, gw),
                                start=(k == 0), stop=(k == KD - 1),
                            )
                    for gi, (g0, gw) in enumerate(groups):
                        ps = pss[gi]
                        if half == 0:
                            t0 = tmp_pool.tile([128, 512], F32, tag="t0")
                            nc.vector.tensor_scalar_min(t0[:, :gw], ps[:, :gw], LIMIT)
                            fg = fg_pool.tile([128, 512], F32, tag="fg")
                            nc.scalar.activation(
                                fg[:, :gw], t0[:, :gw],
                                mybir.ActivationFunctionType.Silu, scale=ALPHA,
                            )
                            if gi == 0:
                                fgs = [fg]
                            else:
                                fgs.append(fg)
                        else:
                            uc = tmp_pool.tile([128, 512], F32, tag="uc")
                            nc.vector.tensor_scalar(
                                uc[:, :gw], ps[:, :gw], LIMIT, -LIMIT,
                                AluOpType.min, AluOpType.max,
                            )
                            # aT = (clip(up)+1) * silu(1.702*min(gate,7))
                            nc.vector.scalar_tensor_tensor(
                                at_sb[:, i * LP + g0: i * LP + g0 + gw],
                                uc[:, :gw], LIN_OFFSET, fgs[gi][:, :gw],
                                AluOpType.add, AluOpType.mult,
                            )

            for dc in range(NDC):
                if dc % 2 == 0 and dc + 2 < NDC:
                    issue_down(j, (dc + 2) // 2)
                dsb = down_tiles[dc // 2]
                sub = (dc % 2) * KI
                last = j == NSLOT - 1 and dc == NDC - 1
                # the very last dc is split so its tail eviction pipelines
                dgroups = groups if not last else \
                    [(0, LP // 2), (LP // 2, LP - LP // 2)]
                ps2s = [psg.tile([128, 512], F32, tag="ps",
                                 name=f"ps2_{j}_{dc}_{gi}")
                        for gi in range(len(dgroups))]
                for k in range(KI):
                    for gi, (g0, gw) in enumerate(dgroups):
                        nc.tensor.matmul(
                            ps2s[gi][:, :gw],
                            lhsT=dsb[:, (sub + k) * 128:(sub + k + 1) * 128],
                            rhs=at_sb[:, k * LP + g0: k * LP + g0 + gw],
                            start=(k == 0), stop=(k == KI - 1),
                        )
                if last:
                    for gi, (g0, gw) in enumerate(dgroups):
                        yt = y_pool.tile([128, lmax], F32, tag="yt")
                        nc.scalar.activation(
                            yt[:, :gw], ps2s[gi][:, :gw],
                            mybir.ActivationFunctionType.Copy,
                        )
                        eng = nc.scalar if gi == 0 else nc.sync
                        eng.dma_start(
                            out=y_d[dc, :, toff[j] + g0: toff[j] + g0 + gw],
                            in_=yt[:, :gw],
                        )
                else:
                    yt = y_pool.tile([128, lmax], F32, tag="yt")
                    for gi, (g0, gw) in enumerate(dgroups):
                        nc.scalar.activation(
                            yt[:, g0:g0 + gw], ps2s[gi][:, :gw],
                            mybir.ActivationFunctionType.Copy,
                        )
                    nc.scalar.dma_start(
                        out=y_d[dc, :, toff[j]: toff[j] + LP],
                        in_=yt[:, :LP],
                    )
            xt_tiles = next_xt
    nc.compile()
    return nc, names


def _route(indices, token_mask, weights):
    """Replicate the reference's permute/capacity semantics on host."""
    idx = np.asarray(indices).astype(np.int64)
    mask = np.asarray(token_mask).astype(bool)
    w = np.asarray(weights).astype(np.float32)
    flat_e = np.where(mask[:, None], idx, -1).ravel()
    w_flat = np.where(flat_e >= 0, w.ravel(), 0.0).astype(np.float32)
    tok = np.repeat(np.arange(N_TOKENS, dtype=np.int64), TOPK)

    per_expert = []  # (first flat_ids, unique token_ids, merged weights)
    for e in range(N_EXPERTS):
        ids = np.nonzero(flat_e == e)[0][:C_REF]
        # a token routed to the same expert k times contributes (w1+..+wk)*y;
        # merge duplicates so each (token, expert) pair is computed once
        ut, first_idx, inv = np.unique(tok[ids], return_index=True,
                                       return_inverse=True)
        uw = np.bincount(inv, weights=w_flat[ids]).astype(np.float32)
        per_expert.append((ids[first_idx], ut, uw))
    return per_expert


def _pack_slots(per_expert):
    """Assign experts to (core, slot); slot stream length = max load in slot."""
    loads = [len(t) for _, t, _ in per_expert]
    order = sorted(range(N_EXPERTS), key=lambda e: -loads[e])
    assign = np.empty((NCORE, NSLOT), np.int64)
    lpads = []
    for j in range(NSLOT):
        col = order[j * NCORE:(j + 1) * NCORE]
        for m in range(NCORE):
            assign[m, j] = col[m]
        lmax = max(loads[e] for e in col)
        lpads.append(max(16, ((lmax + 3) // 4) * 4))
    return assign, tuple(lpads)


def _prepare_core_inputs(x, per_expert, gup, down, assign, lpads):
    x16 = np.ascontiguousarray(np.asarray(x, dtype=np.float32)).astype(BF16)
    gup16 = np.asarray(gup, dtype=np.float32).astype(BF16)
    down16 = np.asarray(down, dtype=np.float32).astype(BF16)
    xt_sizes = [KD * 128 * lp for lp in lpads]
    xt_off = np.concatenate([[0], np.cumsum(xt_sizes)]).tolist()

    in_maps = []
    for m in range(NCORE):
        xt_buf = np.zeros(xt_off[-1], BF16)
        gup_buf = np.empty((NSLOT, 2, KI, 128, KD * 128), BF16)
        down_buf = np.empty((NSLOT, NDC // 2, 128, 2 * KI * 128), BF16)
        for j in range(NSLOT):
            LP = lpads[j]
            e = assign[m, j]
            _, toks, _ = per_expert[e]
            n = len(toks)
            xg = np.zeros((LP, DIM), BF16)
            xg[:n] = x16[toks]
            # p-major slabs: each [128(p), nk, LP] contiguous
            xt = xg.reshape(LP, KD, 128).transpose(2, 1, 0)  # [128, KD, LP]
            bnd = np.concatenate([[0], np.cumsum(XT_SLABS[j])]).tolist()
            xt_buf[xt_off[j]: xt_off[j + 1]] = np.concatenate(
                [np.ascontiguousarray(xt[:, bnd[si]:bnd[si + 1], :]).ravel()
                 for si in range(len(XT_SLABS[j]))])
            for half_gu in (0, 1):
                hm = gup16[e, :, half_gu::2]  # [DIM, INTER] gate or up, deinterleaved
                gup_buf[j, half_gu] = (
                    hm.reshape(KD, 128, KI, 128).transpose(2, 1, 0, 3)
                    .reshape(KI, 128, KD * 128)
                )
            dm = down16[e]  # [INTER, DIM] natural layout, chunked by 128 d-cols
            down_buf[j] = (
                dm.reshape(KI, 128, NDC, 128).transpose(2, 1, 0, 3)
                .reshape(NDC // 2, 2, 128, KI * 128).swapaxes(1, 2)
                .reshape(NDC // 2, 128, 2 * KI * 128)
            )
        in_maps.append({"xt": xt_buf, "gup": gup_buf, "down": down_buf})
    return in_maps


def _run(inputs: dict, trace: bool = False, tmpdir=None):
    from concourse.bass_utils import run_bass_kernel_spmd

    x = inputs["x"]
    gup = inputs["gate_and_up_projs"]
    down = inputs["down_projs"]

    per_expert = _route(inputs["indices"], inputs["token_mask"], inputs["weights"])
    assign, lpads = _pack_slots(per_expert)

    if lpads not in _PROG_CACHE:
        _PROG_CACHE[lpads] = _build_program(lpads)
    nc, names = _PROG_CACHE[lpads]

    core_maps = _prepare_core_inputs(x, per_expert, gup, down, assign, lpads)
    in_maps = [{names[k]: v for k, v in mm.items()} for mm in core_maps]
    res = run_bass_kernel_spmd(
        nc, in_maps, list(range(NCORE)), trace=trace, tmpdir=tmpdir,
    )

    TOT = sum(lpads)
    toff = np.concatenate([[0], np.cumsum(lpads)]).tolist()
    # yT per core: [NDC, 128, TOT] -> [DIM, TOT]
    Y = np.stack([np.asarray(res.results[m][names["y"]]).reshape(DIM, TOT)
                  for m in range(NCORE)])  # [NCORE, DIM, TOT]

    T = N_TOKENS * TOPK
    core_of = np.zeros(T, np.int64)
    col_of = np.zeros(T, np.int64)
    wgt = np.zeros(T, np.float32)
    slot_of = {int(assign[m, j]): (m, j) for m in range(NCORE) for j in range(NSLOT)}
    for e in range(N_EXPERTS):
        ids, _, ws = per_expert[e]
        m, j = slot_of[e]
        core_of[ids] = m
        col_of[ids] = toff[j] + np.arange(len(ids))
        wgt[ids] = ws / ALPHA          # fold silu(a*g)/a into the combine
    contrib = Y[core_of, :, col_of]    # [T, DIM]
    out = (contrib * wgt[:, None]).reshape(N_TOKENS, TOPK, DIM).sum(axis=1)
    return np.ascontiguousarray(out, dtype=np.float32), res


def kernel(**inputs) -> np.ndarray:
    out, _ = _run(inputs, trace=False)
    return out
